# revision 11
# baseline (speedup 1.0000x reference)
"""Trainium2 Bass kernel for nn_ColumnStep (scatter_memory).

Contract: kernel(**inputs) takes FULL unsharded inputs (numpy-convertible),
returns the FULL (B, T, V) float32 output.

Sharding: 8 cores = B(2) x T-query-blocks(4). Each core computes a 512-row
query block. The decay weight d = sigmoid(decay_logit) ~= 0.9526 makes the
anti-causal attention effectively windowed: d^256 ~= 4e-6, so each query
needs only keys within the next ~256 positions. Each core therefore loads a
768-column key window (its own 512 queries + 256 lookahead, zero-padded at
the sequence end) instead of the full 2048-key sequence.

Host folds (data movement / O(K^2) prep only):
  A  = (Wq @ Wk^T) / sqrt(K)        -> scores = gn A gn^T
  M  = (Wv @ Wo) * out_scale * mem_scale -> mem = retr' @ M with raw-gn values
  bu *= write_scale / sqrt(K)
The decay weights are rank-1 per tile pair: d^(w-u-1) = d^(-u%128) (folded
into query columns) * d^(128*delta + w%128 - 1) (per-key-partition vector),
with a constant strict-lower-triangular masked diagonal block. This removes
the O(T*Tq) decay-matrix stream entirely.

All matmuls use float32r operands with >=256-wide outputs (full PE rate).
"""

import sys

for _p in ("/opt/trn_rl_repo", "/root/.axon_site/_ro/trn_rl_repo"):
    if _p not in sys.path:
        sys.path.append(_p)

import numpy as np

import concourse.bass as bass  # noqa: F401  (registers engine mixins)
import concourse.mybir as mybir
from concourse import bacc, tile
from concourse.bass_utils import run_bass_kernel_spmd

F32 = mybir.dt.float32
F32R = mybir.dt.float32r
AF = mybir.ActivationFunctionType
OP = mybir.AluOpType
AX = mybir.AxisListType.X

# Problem shape (hardcoded per spec)
V, K, B, T, NB, INNER = 32000, 256, 2, 2048, 4, 128
EPS = 1.1920929e-07
P = 128
KT = K // P       # 2 tiles along k=256
QF = T // 4       # 512 query rows per core
NQ = QF // P      # 4 query tiles per core
WIN = QF + 256    # 768-col key window per core
NW = WIN // P     # 6 key tiles
NDLT = 2          # off-diagonal decay offsets kept (delta = 1, 2)

_prog_cache = {}


def _build_program():
    nc = bacc.Bacc("TRN2", target_bir_lowering=False, debug=False, num_devices=8)

    gwin_d = nc.dram_tensor("gwin", [KT, P, WIN], F32, kind="ExternalInput")
    # small consts: identity(128) | Msk(128) | ones_col(1) | gw(KT*NB)
    SM = P + P + 1 + KT * NB
    sm_d = nc.dram_tensor("sm", [P, SM], F32R, kind="ExternalInput")
    onesr_d = nc.dram_tensor("onesr", [1, P], F32R, kind="ExternalInput")
    qrow_d = nc.dram_tensor("qrow", [1, QF], F32R, kind="ExternalInput")
    bg_d = nc.dram_tensor("bg", [P, 2 + NB + 3], F32, kind="ExternalInput")
    sel_d = nc.dram_tensor("sel", [NB, NB * P], F32R, kind="ExternalInput")
    packa_d = nc.dram_tensor("packa", [P, KT * K], F32R, kind="ExternalInput")
    # Mt(KT*K) | bd(NB*KT*INNER) | bu(NB*K)
    PB = KT * K + NB * KT * INNER + NB * K
    packb_d = nc.dram_tensor("packb", [P, PB], F32R, kind="ExternalInput")
    o_d = nc.dram_tensor("o", [NQ, P, K], F32, kind="ExternalOutput")

    with tile.TileContext(nc) as tc:
        with (
            nc.allow_low_precision("f32r tiles are bit-identical to f32"),
            tc.tile_pool(name="const", bufs=1) as cp,
            tc.tile_pool(name="persist", bufs=1) as pp,
            tc.tile_pool(name="work", bufs=3) as wp,
            tc.tile_pool(name="stat", bufs=4) as sp,
            tc.tile_pool(name="psM", bufs=3, space="PSUM") as psM,
            tc.tile_pool(name="psS", bufs=2, space="PSUM") as psS,
            tc.tile_pool(name="psR", bufs=1, space="PSUM") as psR,
        ):
            # ---- constant tiles ----
            sm_t = cp.tile([P, SM], F32R, tag="sm")
            ident = sm_t[:, 0:P]
            msk = sm_t[:, P:2 * P]
            ones_col = sm_t[:, 2 * P:2 * P + 1]
            gw_t = sm_t[:, 2 * P + 1:SM].rearrange("p (t n) -> p t n", t=KT)
            ones_row = cp.tile([1, P], F32R, tag="ones_row")
            qrow = cp.tile([1, QF], F32R, tag="qrow")
            bg_t = cp.tile([P, 2 + NB + 3], F32, tag="bg")
            biash = bg_t[:, 0:1]
            gatebT = bg_t[:, 1:2]
            kv = bg_t[:, 2 + NB:2 + NB + 3]
            sel_t = cp.tile([NB, NB * P], F32R, tag="sel")
            eps1 = cp.tile([1, 1], F32, tag="eps1")
            nc.vector.memset(eps1[:], EPS)
            packa = cp.tile([P, KT * K], F32R, tag="packa")
            At = packa[:].rearrange("p (t k) -> p t k", t=KT)
            packb = cp.tile([P, PB], F32R, tag="packb")
            o1 = KT * K
            o2 = o1 + NB * KT * INNER
            Mt = packb[:, 0:o1].rearrange("p (t k) -> p t k", t=KT)
            bd_t = packb[:, o1:o2].rearrange("p (n t h) -> p n t h", n=NB, t=KT)
            bu_t = packb[:, o2:PB].rearrange("p (n k) -> p n k", n=NB)

            # ---- persistent intermediates ----
            gwin = [pp.tile([P, WIN], F32, tag=f"gwin{i}", name=f"gwin{i}")
                    for i in range(KT)]
            gnT = [pp.tile([P, WIN], F32R, tag=f"gnT{i}", name=f"gnT{i}")
                   for i in range(KT)]
            kkT = [pp.tile([P, WIN], F32R, tag=f"kkT{i}", name=f"kkT{i}")
                   for i in range(KT)]
            qs = [pp.tile([P, QF], F32R, tag=f"qs{i}", name=f"qs{i}")
                  for i in range(KT)]
            vt = [pp.tile([P, K], F32R, tag=f"vt{w}", name=f"vt{w}")
                  for w in range(NW)]
            ws_t = [pp.tile([P, 256], F32R, tag=f"ws{t}", name=f"ws{t}")
                    for t in range(4)]
            retr_sb = [pp.tile([P, QF], F32R, tag=f"retr{i}", name=f"retr{i}")
                       for i in range(KT)]
            g2T = [pp.tile([P, QF], F32, tag=f"g2T{i}", name=f"g2T{i}")
                   for i in range(KT)]
            gn2T = [pp.tile([P, QF], F32R, tag=f"gn2T{i}", name=f"gn2T{i}")
                    for i in range(KT)]
            h_sb = [pp.tile([P, QF], F32R, tag=f"h{n}", name=f"h{n}")
                    for n in range(NB)]
            hg = [pp.tile([P, QF], F32R, tag=f"hg{n}", name=f"hg{n}")
                  for n in range(NB)]
            o_sb = [pp.tile([P, K], F32, tag=f"o{q}", name=f"o{q}")
                    for q in range(NQ)]

            # zero blocks of ws tiles that are never written (t=0 right
            # block anti-causal, t=3 left block beyond window)
            nc.vector.tensor_scalar_mul(ws_t[0][:, P:256], msk, 0.0)
            nc.vector.tensor_scalar_mul(ws_t[3][:, 0:P], msk, 0.0)

            # ---- DMAs (order: smalls, g window, A pack, big pack) ----
            nc.sync.dma_start(sm_t[:], sm_d[:])
            nc.sync.dma_start(ones_row[:], onesr_d[:])
            nc.sync.dma_start(qrow[:], qrow_d[:])
            nc.sync.dma_start(bg_t[:], bg_d[:])
            nc.sync.dma_start(sel_t[:], sel_d[:])
            for ki in range(KT):
                nc.sync.dma_start(gwin[ki][:], gwin_d[ki])
            nc.sync.dma_start(packa[:], packa_d[:])
            nc.sync.dma_start(packb[:], packb_d[:])

            CH = ((0, 512), (512, WIN))  # psum-sized column chunks

            # ---- norm1 over the 768-col window ----
            sq = wp.tile([P, KT, WIN], F32R, tag="sq", bufs=1)
            for ki in range(KT):
                nc.vector.tensor_mul(sq[:, ki, :], gwin[ki][:], gwin[ki][:])
            rrow = sp.tile([1, WIN], F32R, tag="rrow")
            for c0, c1 in CH:
                w = c1 - c0
                cs = psM.tile([1, 512], F32, tag="mm")
                for ki in range(KT):
                    nc.tensor.matmul(cs[:1, :w], ones_col, sq[:, ki, c0:c1],
                                     start=(ki == 0), stop=(ki == KT - 1))
                rt = sp.tile([1, 512], F32R, tag="rt")
                nc.scalar.activation(rt[:1, :w], cs[:1, :w], AF.Sqrt,
                                     bias=eps1[:], scale=1.0 / K)
                nc.vector.reciprocal(rrow[:1, c0:c1], rt[:1, :w])
            for c0, c1 in CH:
                w = c1 - c0
                bc = psM.tile([P, 512], F32, tag="mm")
                nc.tensor.matmul(bc[:, :w], ones_row[:], rrow[:1, c0:c1],
                                 start=True, stop=True)
                for ki in range(KT):
                    nc.vector.tensor_mul(gnT[ki][:, c0:c1], gwin[ki][:, c0:c1],
                                         bc[:, :w])

            # ---- scaled queries: qs = gnT[:, :512] * d^-(u%128) ----
            qbc = psM.tile([P, QF], F32, tag="mm")
            nc.tensor.matmul(qbc[:], ones_row[:], qrow[:], start=True, stop=True)
            for ki in range(KT):
                nc.vector.tensor_mul(qs[ki][:], gnT[ki][:, 0:QF], qbc[:])

            # ---- kk~ = A @ gn (k-major) ----
            for ko in range(KT):
                for c0, c1 in CH:
                    w = c1 - c0
                    ps = psM.tile([P, 512], F32, tag="mm")
                    for ki in range(KT):
                        nc.tensor.matmul(ps[:, :w], At[:, ki, ko * P:(ko + 1) * P],
                                         gnT[ki][:, c0:c1],
                                         start=(ki == 0), stop=(ki == KT - 1))
                    nc.scalar.copy(kkT[ko][:, c0:c1], ps[:, :w])

            # ---- value transposes: vt[w] = gn window tile, token-major ----
            for wt in range(NW):
                for ki in range(KT):
                    tp = psS.tile([P, P], F32R, tag="sc")
                    nc.tensor.transpose(tp[:], gnT[ki][:, wt * P:(wt + 1) * P],
                                        ident)
                    nc.scalar.copy(vt[wt][:, ki * P:(ki + 1) * P], tp[:])

            # ---- windowed decayed attention ----
            retr_ps = [psR.tile([P, QF], F32, tag=f"rps{kt}", name=f"rps{kt}")
                       for kt in range(KT)]
            for hh in range(2):
                hc = slice(hh * 256, hh * 256 + 256)
                for t in range(4):
                    wt = 2 * hh + t
                    sc = psS.tile([P, 256], F32, tag="sc")
                    for ko in range(KT):
                        nc.tensor.matmul(sc[:], kkT[ko][:, wt * P:(wt + 1) * P],
                                         qs[ko][:, hc],
                                         start=(ko == 0), stop=(ko == KT - 1))
                    # decay-weight application per 128-col query block
                    if t == 0:
                        nc.vector.scalar_tensor_tensor(
                            ws_t[0][:, 0:P], sc[:, 0:P], kv[:, 0:1], msk,
                            op0=OP.mult, op1=OP.mult)
                    elif t == 1:
                        nc.vector.tensor_scalar_mul(
                            ws_t[1][:, 0:P], sc[:, 0:P], kv[:, 1:2])
                        nc.vector.scalar_tensor_tensor(
                            ws_t[1][:, P:256], sc[:, P:256], kv[:, 0:1], msk,
                            op0=OP.mult, op1=OP.mult)
                    elif t == 2:
                        nc.vector.tensor_scalar_mul(
                            ws_t[2][:, 0:P], sc[:, 0:P], kv[:, 2:3])
                        nc.vector.tensor_scalar_mul(
                            ws_t[2][:, P:256], sc[:, P:256], kv[:, 1:2])
                    else:
                        nc.vector.tensor_scalar_mul(
                            ws_t[3][:, P:256], sc[:, P:256], kv[:, 2:3])
                    for kt in range(KT):
                        nc.tensor.matmul(
                            retr_ps[kt][:, hc], vt[wt][:, kt * P:(kt + 1) * P],
                            ws_t[t][:], start=(t == 0), stop=(t == 3),
                            skip_group_check=True)
            for kt in range(KT):
                nc.scalar.copy(retr_sb[kt][:], retr_ps[kt][:])

            # ---- mem = retr' @ M; residual; norm2 ----
            for ko in range(KT):
                ps = psM.tile([P, QF], F32, tag="mm")
                for ki in range(KT):
                    nc.tensor.matmul(ps[:], Mt[:, ki, ko * P:(ko + 1) * P],
                                     retr_sb[ki][:],
                                     start=(ki == 0), stop=(ki == KT - 1))
                nc.vector.tensor_add(g2T[ko][:], gwin[ko][:, 0:QF], ps[:])

            sq2 = wp.tile([P, KT, QF], F32R, tag="sq2", bufs=1)
            for ki in range(KT):
                nc.vector.tensor_mul(sq2[:, ki, :], g2T[ki][:], g2T[ki][:])
            cs2 = psM.tile([1, QF], F32, tag="mm")
            for ki in range(KT):
                nc.tensor.matmul(cs2[:], ones_col, sq2[:, ki, :],
                                 start=(ki == 0), stop=(ki == KT - 1))
            rt2 = sp.tile([1, QF], F32R, tag="rt2")
            nc.scalar.activation(rt2[:], cs2[:], AF.Sqrt, bias=eps1[:],
                                 scale=1.0 / K)
            rrow2 = sp.tile([1, QF], F32R, tag="rrow2")
            nc.vector.reciprocal(rrow2[:], rt2[:])
            bc2 = psM.tile([P, QF], F32, tag="mm")
            nc.tensor.matmul(bc2[:], ones_row[:], rrow2[:], start=True, stop=True)
            for ki in range(KT):
                nc.vector.tensor_mul(gn2T[ki][:], g2T[ki][:], bc2[:])

            # ---- dendritic MLP: h = gelu(bd^T gn2 + bias) ----
            for n in range(NB):
                hp = psM.tile([P, QF], F32, tag="mm")
                for ki in range(KT):
                    nc.tensor.matmul(hp[:], bd_t[:, n, ki, :], gn2T[ki][:],
                                     start=(ki == 0), stop=(ki == KT - 1))
                nc.scalar.activation(h_sb[n][:], hp[:], AF.Gelu, bias=biash)

            # ---- gates: softmax over branches, computed transposed ----
            zt = psM.tile([NB, QF], F32, tag="mm")
            for ki in range(KT):
                nc.tensor.matmul(zt[:], gw_t[:, ki, :], gn2T[ki][:],
                                 start=(ki == 0), stop=(ki == KT - 1))
            exT = sp.tile([NB, QF], F32R, tag="exT")
            nc.scalar.activation(exT[:], zt[:], AF.Exp, bias=gatebT[0:NB, :])
            gsum = psM.tile([1, QF], F32, tag="mm")
            nc.tensor.matmul(gsum[:], ones_col[0:NB, :], exT[:],
                             start=True, stop=True)
            rsum = sp.tile([1, QF], F32R, tag="rsum")
            nc.vector.reciprocal(rsum[:], gsum[:])
            rb = psM.tile([NB, QF], F32, tag="mm")
            nc.tensor.matmul(rb[:], ones_row[:, 0:NB], rsum[:],
                             start=True, stop=True)
            exn = sp.tile([NB, QF], F32R, tag="exn")
            nc.vector.tensor_mul(exn[:], exT[:], rb[:])

            # hg[n] = h[n] * broadcast(gates row n)
            for n in range(NB):
                gb = psM.tile([P, QF], F32, tag="mm")
                nc.tensor.matmul(gb[:], sel_t[:, n * P:(n + 1) * P], exn[:],
                                 start=True, stop=True)
                nc.vector.tensor_mul(hg[n][:], h_sb[n][:], gb[:])

            # ---- o[qt] = sum_n hg[n][:, qt]^T @ bu[n]  (bu pre-scaled) ----
            for qt in range(NQ):
                op_ps = psM.tile([P, K], F32, tag="mm")
                for n in range(NB):
                    nc.tensor.matmul(op_ps[:], hg[n][:, qt * P:(qt + 1) * P],
                                     bu_t[:, n, :],
                                     start=(n == 0), stop=(n == NB - 1))
                nc.scalar.copy(o_sb[qt][:], op_ps[:])
                nc.sync.dma_start(o_d[qt], o_sb[qt][:])

    nc.compile()
    return nc


def kernel(**inputs):
    x = np.asarray(inputs["x"], np.float32)
    Wq = np.asarray(inputs["Wq"], np.float32)
    Wk = np.asarray(inputs["Wk"], np.float32)
    Wv = np.asarray(inputs["Wv"], np.float32)
    Wo = np.asarray(inputs["Wo"], np.float32)
    decay_logit = np.float32(np.asarray(inputs["decay_logit"]).reshape(()))
    out_scale = np.float32(np.asarray(inputs["out_scale"]).reshape(()))
    mem_scale = np.float32(np.asarray(inputs["mem_scale"]).reshape(-1)[0])
    branch_down = np.asarray(inputs["branch_down"], np.float32)
    branch_up = np.asarray(inputs["branch_up"], np.float32)
    mlp_bias = np.asarray(inputs["mlp_bias"], np.float32)
    gate_W = np.asarray(inputs["gate_W"], np.float32)
    gate_b = np.asarray(inputs["gate_b"], np.float32)
    write_scale = np.float32(np.asarray(inputs["write_scale"]).reshape(()))
    read_idx = np.asarray(inputs["read_indices"]).astype(np.int64)
    write_idx = np.asarray(inputs["write_indices"]).astype(np.int64)

    g = np.take(x, read_idx, axis=2)  # (B, T, K) host gather

    decay = np.float32(1.0) / (np.float32(1.0) + np.exp(-decay_logit,
                                                        dtype=np.float32))
    s_qk = np.float32(1.0 / np.sqrt(np.float32(K)))
    c_mem = np.float32(out_scale * mem_scale)
    s_out = np.float32(write_scale * s_qk)

    nc = _prog_cache.get("prog")
    if nc is None:
        nc = _build_program()
        _prog_cache["prog"] = nc

    # host folds
    A = (Wq @ Wk.T) * s_qk                       # (K, K)
    M = (Wv @ Wo) * c_mem                        # (K, K)
    At_pack = np.ascontiguousarray(
        A.T.reshape(KT, P, K).transpose(1, 0, 2)).reshape(P, KT * K)
    Mt_pack = np.ascontiguousarray(
        M.reshape(KT, P, K).transpose(1, 0, 2)).reshape(P, KT * K)
    bd_pack = np.ascontiguousarray(
        branch_down.reshape(NB, KT, P, INNER).transpose(2, 0, 1, 3)
    ).reshape(P, -1)
    bu_pack = (branch_up.transpose(1, 0, 2) * s_out).reshape(P, -1)
    packb = np.concatenate([Mt_pack, bd_pack, bu_pack], axis=1).astype(np.float32)

    # decay constants
    pidx = np.arange(P, dtype=np.float32)
    ln_d = np.log(decay)
    kvs = np.stack([np.exp((pidx - 1.0) * ln_d),
                    np.exp((P + pidx - 1.0) * ln_d),
                    np.exp((2 * P + pidx - 1.0) * ln_d)], axis=1)  # (P, 3)
    qrow = np.exp(-(np.arange(QF, dtype=np.float32) % P) * ln_d).reshape(1, QF)
    msk = np.tril(np.ones((P, P), np.float32), -1)  # msk[p,f]=1 iff p>f
    ident = np.eye(P, dtype=np.float32)
    gw = gate_W.reshape(KT, P, NB).transpose(1, 0, 2).reshape(P, KT * NB)
    sm = np.concatenate([ident, msk,
                         np.ones((P, 1), np.float32), gw], axis=1)
    gcol = np.zeros((P, 1), np.float32)
    gcol[:NB, 0] = gate_b
    bg = np.concatenate([mlp_bias.reshape(P, 1), gcol,
                         np.zeros((P, NB), np.float32),
                         kvs.astype(np.float32)], axis=1)
    bg = np.ascontiguousarray(bg, dtype=np.float32)

    common = {
        "sm": np.ascontiguousarray(sm, np.float32),
        "onesr": np.ones((1, P), np.float32),
        "qrow": np.ascontiguousarray(qrow, np.float32),
        "bg": bg,
        "sel": np.ascontiguousarray(
            np.kron(np.eye(NB, dtype=np.float32), np.ones((1, P), np.float32))),
        "packa": At_pack.astype(np.float32),
        "packb": packb,
    }

    in_maps = []
    for c in range(8):
        b, qc = divmod(c, NQ)
        j0 = qc * QF
        j1 = min(j0 + WIN, T)
        gwin = np.zeros((KT, P, WIN), np.float32)
        blk = np.ascontiguousarray(g[b, j0:j1].T)  # (K, j1-j0)
        gwin[:, :, : j1 - j0] = blk.reshape(KT, P, j1 - j0)
        in_maps.append(dict(common, gwin=gwin))

    res = run_bass_kernel_spmd(nc, in_maps, list(range(8)))

    out = np.zeros((B, T, V), np.float32)
    for c in range(8):
        b, qc = divmod(c, NQ)
        oc = res.results[c]["o"].reshape(QF, K)
        out[b, qc * QF:(qc + 1) * QF, :][:, write_idx] = oc
    return out


# revision 15
# speedup vs baseline: 1.1456x; 1.1456x over previous
"""Trainium2 Bass kernel for nn_ColumnStep (scatter_memory).

Contract: kernel(**inputs) takes FULL unsharded inputs (numpy-convertible),
returns the FULL (B, T, V) float32 output.

Sharding: 8 cores = B(2) x T-query-blocks(4). Each core computes a 512-row
query block. The decay d = sigmoid(decay_logit) ~= 0.9526 makes the
anti-causal attention effectively windowed: d^256 ~= 4e-6, so each query
needs only keys within the next ~256 positions. Each core loads a 768-column
key window (its 512 queries + 256 lookahead, zero-padded at the sequence
end) instead of the full 2048-key sequence.

Host folds: A = (Wq Wk^T)/sqrt(K), M = (Wv Wo)*out_scale*mem_scale,
bu *= write_scale/sqrt(K). Decay weights are rank-1 per tile pair:
d^(w-u-1) = d^-(u%128) (query columns) * d^(128*delta + w%128 - 1)
(key-partition vector), plus one constant masked diagonal block.

The first rms-norm is never materialized in k-major form: the kernel
computes raw-data transposes and kk~ = A @ g_raw immediately off the DMA,
derives per-token inverse norms once per layout (row form via ones-matmul
for queries, column form via tensor_tensor_reduce on the transposes for
keys/values), and folds them into the query scaling, the decay vectors,
and the value-copy scale. All matmuls use f32r with >=256-wide outputs.
"""

import sys

for _p in ("/opt/trn_rl_repo", "/root/.axon_site/_ro/trn_rl_repo"):
    if _p not in sys.path:
        sys.path.append(_p)

import numpy as np

import concourse.bass as bass  # noqa: F401  (registers engine mixins)
import concourse.mybir as mybir
from concourse import bacc, tile
from concourse.bass_utils import run_bass_kernel_spmd

F32 = mybir.dt.float32
F32R = mybir.dt.float32r
AF = mybir.ActivationFunctionType
OP = mybir.AluOpType
AX = mybir.AxisListType.X

V, K, B, T, NB, INNER = 32000, 256, 2, 2048, 4, 128
EPS = 1.1920929e-07
P = 128
KT = K // P       # 2 tiles along k=256
QF = T // 4       # 512 query rows per core
NQ = QF // P      # 4 query tiles per core
WIN = QF + 256    # 768-col key window per core
NW = WIN // P     # 6 key tiles

_prog_cache = {}


def _build_program():
    nc = bacc.Bacc("TRN2", target_bir_lowering=False, debug=False, num_devices=8)

    gwin_d = nc.dram_tensor("gwin", [KT, P, WIN], F32R, kind="ExternalInput")
    # sm: identity(128) | Msk(128) | ones_col(1) | gw(KT*NB)
    SM = P + P + 1 + KT * NB
    sm_d = nc.dram_tensor("sm", [P, SM], F32R, kind="ExternalInput")
    # rows: [4, 128(ones_row) | 512(qrow) | 512(sel)]
    RW = P + QF + NB * P
    rows_d = nc.dram_tensor("rows", [NB, RW], F32R, kind="ExternalInput")
    # bg (f32): biash(1) | gatebT(1) | kv(3)
    bg_d = nc.dram_tensor("bg", [P, 5], F32, kind="ExternalInput")
    packa_d = nc.dram_tensor("packa", [P, KT * K], F32R, kind="ExternalInput")
    PB = KT * K + NB * KT * INNER + NB * K
    packb_d = nc.dram_tensor("packb", [P, PB], F32R, kind="ExternalInput")
    o_d = nc.dram_tensor("o", [P, NQ * K], F32, kind="ExternalOutput")

    with tile.TileContext(nc) as tc:
        with (
            nc.allow_low_precision("f32r tiles are bit-identical to f32"),
            tc.tile_pool(name="const", bufs=1) as cp,
            tc.tile_pool(name="persist", bufs=1) as pp,
            tc.tile_pool(name="stat", bufs=4) as sp,
            tc.tile_pool(name="psM", bufs=3, space="PSUM") as psM,
            tc.tile_pool(name="psS", bufs=3, space="PSUM") as psS,
            tc.tile_pool(name="psR", bufs=1, space="PSUM") as psR,
        ):
            # ---- constants ----
            sm_t = cp.tile([P, SM], F32R, tag="sm")
            ident = sm_t[:, 0:P]
            msk = sm_t[:, P:2 * P]
            ones_col = sm_t[:, 2 * P:2 * P + 1]
            gw_t = sm_t[:, 2 * P + 1:SM].rearrange("p (t n) -> p t n", t=KT)
            rows_t = cp.tile([NB, RW], F32R, tag="rows")
            ones_row = rows_t[0:1, 0:P]
            qrow = rows_t[0:1, P:P + QF]
            sel_t = rows_t[:, P + QF:RW]
            bg_t = cp.tile([P, 5], F32, tag="bg")
            biash = bg_t[:, 0:1]
            gatebT = bg_t[0:NB, 1:2]
            kv = bg_t[:, 2:5]
            eps1 = cp.tile([1, 1], F32, tag="eps1")
            nc.vector.memset(eps1[:], EPS)
            eps128 = cp.tile([P, 1], F32, tag="eps128")
            nc.vector.memset(eps128[:], EPS)
            packa = cp.tile([P, KT * K], F32R, tag="packa")
            At = packa[:].rearrange("p (t k) -> p t k", t=KT)
            packb = cp.tile([P, PB], F32R, tag="packb")
            o1 = KT * K
            o2 = o1 + NB * KT * INNER
            Mt = packb[:, 0:o1].rearrange("p (t k) -> p t k", t=KT)
            bd_t = packb[:, o1:o2].rearrange("p (n t h) -> p n t h", n=NB, t=KT)
            bu_t = packb[:, o2:PB].rearrange("p (n k) -> p n k", n=NB)

            # ---- persistent intermediates ----
            gwin = [pp.tile([P, WIN], F32R, tag=f"gwin{i}", name=f"gwin{i}")
                    for i in range(KT)]
            sqq = pp.tile([P, KT, QF], F32R, tag="sqq", name="sqq")
            kkT = [pp.tile([P, WIN], F32R, tag=f"kkT{i}", name=f"kkT{i}")
                   for i in range(KT)]
            qs = [pp.tile([P, QF], F32R, tag=f"qs{i}", name=f"qs{i}")
                  for i in range(KT)]
            vt = [pp.tile([P, K], F32R, tag=f"vt{w}", name=f"vt{w}")
                  for w in range(NW)]
            ssq = pp.tile([P, NW], F32, tag="ssq", name="ssq")
            rcol = pp.tile([P, NW], F32, tag="rcol", name="rcol")
            kvr = pp.tile([P, NQ], F32, tag="kvr", name="kvr")
            qcomb = pp.tile([1, QF], F32R, tag="qcomb", name="qcomb")
            ws_t = [pp.tile([P, 256], F32R, tag=f"ws{t}", name=f"ws{t}")
                    for t in range(4)]
            retr_sb = [pp.tile([P, QF], F32R, tag=f"retr{i}", name=f"retr{i}")
                       for i in range(KT)]
            g2T = [pp.tile([P, QF], F32R, tag=f"g2T{i}", name=f"g2T{i}")
                   for i in range(KT)]
            sq2 = pp.tile([P, KT, QF], F32R, tag="sq2", name="sq2")
            gn2T = [pp.tile([P, QF], F32R, tag=f"gn2T{i}", name=f"gn2T{i}")
                    for i in range(KT)]
            h_sb = [pp.tile([P, QF], F32R, tag=f"h{n}", name=f"h{n}")
                    for n in range(NB)]
            exT = pp.tile([NB, QF], F32R, tag="exT", name="exT")
            exn = pp.tile([NB, QF], F32R, tag="exn", name="exn")
            hg = [pp.tile([P, QF], F32R, tag=f"hg{n}", name=f"hg{n}")
                  for n in range(NB)]
            o_sb = pp.tile([P, NQ * K], F32, tag="o_sb", name="o_sb")

            rrowq = sp.tile([1, QF], F32R, tag="rrowq")
            rrow2 = sp.tile([1, QF], F32R, tag="rrow2")
            rsum = sp.tile([1, QF], F32R, tag="rsum")

            # ws blocks never written elsewhere (t0 right anti-causal,
            # t3 left beyond window): zero once
            nc.vector.tensor_scalar_mul(ws_t[0][:, P:256], msk, 0.0)
            nc.vector.tensor_scalar_mul(ws_t[3][:, 0:P], msk, 0.0)

            # ---- DMAs: g window first, then consts/params ----
            for ki in range(KT):
                nc.sync.dma_start(gwin[ki][:], gwin_d[ki])
            nc.sync.dma_start(sm_t[:], sm_d[:])
            nc.sync.dma_start(rows_t[:], rows_d[:])
            nc.sync.dma_start(bg_t[:], bg_d[:])
            nc.sync.dma_start(packa[:], packa_d[:])
            nc.sync.dma_start(packb[:], packb_d[:])

            # 1) raw transposes of the window (value path + column
            # norms); copies stay raw -- the key norm is folded squared into
            # the ws decay vectors (scores r_w and values r_w)
            sqs = pp.tile([P, 256], F32, tag="sqs", name="sqs")
            for wt in range(NW):
                tp = psS.tile([P, 512], F32R, tag="sc")
                for ki in range(KT):
                    nc.tensor.transpose(tp[:, ki * P:(ki + 1) * P],
                                        gwin[ki][:, wt * P:(wt + 1) * P], ident)
                nc.scalar.copy(vt[wt][:], tp[:, 0:256])
                nc.vector.tensor_mul(sqs[:], vt[wt][:], vt[wt][:])
                nc.vector.reduce_sum(ssq[:, wt:wt + 1], sqs[:], axis=AX)

            # DVE: query-row squares (for the row-form norm)
            for ki in range(KT):
                nc.vector.tensor_mul(sqq[:, ki, :], gwin[ki][:, 0:QF],
                                     gwin[ki][:, 0:QF])

            # 2) query-row sum of squares; rrowq = rsqrt(mean+eps)
            csq = psM.tile([1, QF], F32, tag="mm")
            for ki in range(KT):
                nc.tensor.matmul(csq[:], ones_col, sqq[:, ki, :],
                                 start=(ki == 0), stop=(ki == KT - 1))
            rtq = sp.tile([1, QF], F32R, tag="rtq")
            nc.scalar.activation(rtq[:], csq[:], AF.Sqrt, bias=eps1[:],
                                 scale=1.0 / K)
            nc.vector.reciprocal(rrowq[:], rtq[:])
            rts = sp.tile([P, NW], F32, tag="rts")
            nc.scalar.activation(rts[:], ssq[:], AF.Sqrt, bias=eps128[:],
                                 scale=1.0 / K)
            nc.vector.reciprocal(rcol[:], rts[:])
            rcol2 = pp.tile([P, NW], F32, tag="rcol2", name="rcol2")
            nc.vector.tensor_mul(rcol2[:], rcol[:], rcol[:])
            nc.vector.tensor_scalar_mul(kvr[:], rcol2[:, 0:NQ], kv[:, 0:1])
            nc.vector.tensor_mul(qcomb[:], qrow, rrowq[:])

            # 3) kk~ = A @ g_raw (k-major), PSUM -> SBUF on Act
            for ko in range(KT):
                for c0, c1 in ((0, 512), (512, WIN)):
                    w = c1 - c0
                    ps = psM.tile([P, 512], F32, tag="mm")
                    for ki in range(KT):
                        nc.tensor.matmul(ps[:, :w], At[:, ki, ko * P:(ko + 1) * P],
                                         gwin[ki][:, c0:c1],
                                         start=(ki == 0), stop=(ki == KT - 1))
                    nc.scalar.copy(kkT[ko][:, c0:c1], ps[:, :w])

            # 4) scaled queries: qs = g_raw * broadcast(qrow * rrowq)
            qbc = psM.tile([P, QF], F32, tag="mm")
            nc.tensor.matmul(qbc[:], ones_row, qcomb[:], start=True, stop=True)
            for ki in range(KT):
                nc.vector.tensor_mul(qs[ki][:], gwin[ki][:, 0:QF], qbc[:])

            # 5) windowed decayed attention
            retr_ps = [psR.tile([P, QF], F32, tag=f"rps{kt}", name=f"rps{kt}")
                       for kt in range(KT)]
            for hh in range(2):
                hc = slice(hh * 256, hh * 256 + 256)
                for t in range(4):
                    wt = 2 * hh + t
                    sc = psS.tile([P, 512], F32, tag="sc")
                    for ko in range(KT):
                        nc.tensor.matmul(sc[:, 0:256],
                                         kkT[ko][:, wt * P:(wt + 1) * P],
                                         qs[ko][:, hc],
                                         start=(ko == 0), stop=(ko == KT - 1))
                    # DVE: decay + key-norm application per 128-col block
                    if t == 0:
                        nc.vector.scalar_tensor_tensor(
                            ws_t[0][:, 0:P], sc[:, 0:P], kvr[:, wt:wt + 1],
                            msk, op0=OP.mult, op1=OP.mult)
                    elif t == 1:
                        nc.vector.tensor_scalar(
                            ws_t[1][:, 0:P], sc[:, 0:P], kv[:, 1:2],
                            rcol2[:, wt:wt + 1], op0=OP.mult, op1=OP.mult)
                        nc.vector.scalar_tensor_tensor(
                            ws_t[1][:, P:256], sc[:, P:256], kvr[:, wt:wt + 1],
                            msk, op0=OP.mult, op1=OP.mult)
                    elif t == 2:
                        nc.vector.tensor_scalar(
                            ws_t[2][:, 0:P], sc[:, 0:P], kv[:, 2:3],
                            rcol2[:, wt:wt + 1], op0=OP.mult, op1=OP.mult)
                        nc.vector.tensor_scalar(
                            ws_t[2][:, P:256], sc[:, P:256], kv[:, 1:2],
                            rcol2[:, wt:wt + 1], op0=OP.mult, op1=OP.mult)
                    else:
                        nc.vector.tensor_scalar(
                            ws_t[3][:, P:256], sc[:, P:256], kv[:, 2:3],
                            rcol2[:, wt:wt + 1], op0=OP.mult, op1=OP.mult)
                for t in range(4):
                    wt = 2 * hh + t
                    for kt in range(KT):
                        nc.tensor.matmul(
                            retr_ps[kt][:, hc], vt[wt][:, kt * P:(kt + 1) * P],
                            ws_t[t][:], start=(t == 0), stop=(t == 3),
                            skip_group_check=True)
            for kt in range(KT):
                nc.scalar.copy(retr_sb[kt][:], retr_ps[kt][:])

            # 6) mem = retr' @ M; residual (r_u already folded via qs)
            for ko in range(KT):
                ps = psM.tile([P, QF], F32, tag="mm")
                for ki in range(KT):
                    nc.tensor.matmul(ps[:], Mt[:, ki, ko * P:(ko + 1) * P],
                                     retr_sb[ki][:],
                                     start=(ki == 0), stop=(ki == KT - 1))
                nc.vector.tensor_add(g2T[ko][:], gwin[ko][:, 0:QF], ps[:])

            # 7) norm2 (row form)
            for ki in range(KT):
                nc.vector.tensor_mul(sq2[:, ki, :], g2T[ki][:], g2T[ki][:])
            cs2 = psM.tile([1, QF], F32, tag="mm")
            for ki in range(KT):
                nc.tensor.matmul(cs2[:], ones_col, sq2[:, ki, :],
                                 start=(ki == 0), stop=(ki == KT - 1))
            rt2 = sp.tile([1, QF], F32R, tag="rt2")
            nc.scalar.activation(rt2[:], cs2[:], AF.Sqrt, bias=eps1[:],
                                 scale=1.0 / K)
            nc.vector.reciprocal(rrow2[:], rt2[:])
            bc2 = psM.tile([P, QF], F32, tag="mm")
            nc.tensor.matmul(bc2[:], ones_row, rrow2[:], start=True, stop=True)
            for ki in range(KT):
                nc.vector.tensor_mul(gn2T[ki][:], g2T[ki][:], bc2[:])

            # 8) gates (transposed softmax) before MLP so the Act exp table
            # load overlaps the h matmuls
            zt = psM.tile([NB, QF], F32, tag="mm")
            for ki in range(KT):
                nc.tensor.matmul(zt[:], gw_t[:, ki, :], gn2T[ki][:],
                                 start=(ki == 0), stop=(ki == KT - 1))
            nc.scalar.activation(exT[:], zt[:], AF.Exp, bias=gatebT)
            gsum = psM.tile([1, QF], F32, tag="mm")
            nc.tensor.matmul(gsum[:], ones_col[0:NB, :], exT[:],
                             start=True, stop=True)
            nc.vector.reciprocal(rsum[:], gsum[:])
            rb = psM.tile([NB, QF], F32, tag="mm")
            nc.tensor.matmul(rb[:], ones_row[:, 0:NB], rsum[:],
                             start=True, stop=True)
            nc.vector.tensor_mul(exn[:], exT[:], rb[:])

            # 9) dendritic MLP
            for n in range(NB):
                hp = psM.tile([P, QF], F32, tag="mm")
                for ki in range(KT):
                    nc.tensor.matmul(hp[:], bd_t[:, n, ki, :], gn2T[ki][:],
                                     start=(ki == 0), stop=(ki == KT - 1))
                nc.scalar.activation(h_sb[n][:], hp[:], AF.Gelu, bias=biash)

            # 10) gate-weighted branch mix
            for n in range(NB):
                gb = psM.tile([P, QF], F32, tag="mm")
                nc.tensor.matmul(gb[:], sel_t[:, n * P:(n + 1) * P], exn[:],
                                 start=True, stop=True)
                nc.vector.tensor_mul(hg[n][:], h_sb[n][:], gb[:])

            # 11) output projection (bu pre-scaled by write_scale/sqrt(K))
            for qt in range(NQ):
                op_ps = psS.tile([P, 512], F32, tag="sc")
                for n in range(NB):
                    nc.tensor.matmul(op_ps[:, 0:K],
                                     hg[n][:, qt * P:(qt + 1) * P],
                                     bu_t[:, n, :],
                                     start=(n == 0), stop=(n == NB - 1))
                if qt % 2 == 0:
                    nc.scalar.copy(o_sb[:, qt * K:(qt + 1) * K], op_ps[:, 0:K])
                else:
                    nc.vector.tensor_copy(o_sb[:, qt * K:(qt + 1) * K],
                                          op_ps[:, 0:K])
            nc.sync.dma_start(o_d[:], o_sb[:])

    nc.compile()
    return nc


def kernel(**inputs):
    x = np.asarray(inputs["x"], np.float32)
    Wq = np.asarray(inputs["Wq"], np.float32)
    Wk = np.asarray(inputs["Wk"], np.float32)
    Wv = np.asarray(inputs["Wv"], np.float32)
    Wo = np.asarray(inputs["Wo"], np.float32)
    decay_logit = np.float32(np.asarray(inputs["decay_logit"]).reshape(()))
    out_scale = np.float32(np.asarray(inputs["out_scale"]).reshape(()))
    mem_scale = np.float32(np.asarray(inputs["mem_scale"]).reshape(-1)[0])
    branch_down = np.asarray(inputs["branch_down"], np.float32)
    branch_up = np.asarray(inputs["branch_up"], np.float32)
    mlp_bias = np.asarray(inputs["mlp_bias"], np.float32)
    gate_W = np.asarray(inputs["gate_W"], np.float32)
    gate_b = np.asarray(inputs["gate_b"], np.float32)
    write_scale = np.float32(np.asarray(inputs["write_scale"]).reshape(()))
    read_idx = np.asarray(inputs["read_indices"]).astype(np.int64)
    write_idx = np.asarray(inputs["write_indices"]).astype(np.int64)

    g = np.take(x, read_idx, axis=2)  # (B, T, K) host gather

    decay = np.float32(1.0) / (np.float32(1.0) + np.exp(-decay_logit,
                                                        dtype=np.float32))
    s_qk = np.float32(1.0 / np.sqrt(np.float32(K)))
    c_mem = np.float32(out_scale * mem_scale)
    s_out = np.float32(write_scale * s_qk)

    nc = _prog_cache.get("prog")
    if nc is None:
        nc = _build_program()
        _prog_cache["prog"] = nc

    A = (Wq @ Wk.T) * s_qk
    M = (Wv @ Wo) * c_mem
    At_pack = np.ascontiguousarray(
        A.T.reshape(KT, P, K).transpose(1, 0, 2)).reshape(P, KT * K)
    Mt_pack = np.ascontiguousarray(
        M.reshape(KT, P, K).transpose(1, 0, 2)).reshape(P, KT * K)
    bd_pack = np.ascontiguousarray(
        branch_down.reshape(NB, KT, P, INNER).transpose(2, 0, 1, 3)
    ).reshape(P, -1)
    bu_pack = (branch_up.transpose(1, 0, 2) * s_out).reshape(P, -1)
    packb = np.concatenate([Mt_pack, bd_pack, bu_pack], axis=1).astype(np.float32)

    pidx = np.arange(P, dtype=np.float32)
    ln_d = float(np.log(decay))
    kvs = np.stack([np.exp((pidx - 1.0) * ln_d),
                    np.exp((P + pidx - 1.0) * ln_d),
                    np.exp((2 * P + pidx - 1.0) * ln_d)], axis=1)
    qrow = np.exp(-(np.arange(QF, dtype=np.float32) % P) * ln_d)
    msk = np.tril(np.ones((P, P), np.float32), -1)  # msk[p,f]=1 iff p>f
    ident = np.eye(P, dtype=np.float32)
    gw = gate_W.reshape(KT, P, NB).transpose(1, 0, 2).reshape(P, KT * NB)
    sm = np.concatenate([ident, msk, np.ones((P, 1), np.float32), gw], axis=1)
    rows = np.zeros((NB, P + QF + NB * P), np.float32)
    rows[0, :P] = 1.0
    rows[0, P:P + QF] = qrow
    rows[:, P + QF:] = np.kron(np.eye(NB, dtype=np.float32),
                               np.ones((1, P), np.float32))
    gcol = np.zeros((P, 1), np.float32)
    gcol[:NB, 0] = gate_b
    bg = np.concatenate([mlp_bias.reshape(P, 1), gcol,
                         kvs.astype(np.float32)], axis=1)

    common = {
        "sm": np.ascontiguousarray(sm, np.float32),
        "rows": np.ascontiguousarray(rows),
        "bg": np.ascontiguousarray(bg, np.float32),
        "packa": At_pack.astype(np.float32),
        "packb": packb,
    }

    in_maps = []
    for c in range(8):
        b, qc = divmod(c, NQ)
        j0 = qc * QF
        j1 = min(j0 + WIN, T)
        gwin = np.zeros((KT, P, WIN), np.float32)
        blk = np.ascontiguousarray(g[b, j0:j1].T)
        gwin[:, :, : j1 - j0] = blk.reshape(KT, P, j1 - j0)
        in_maps.append(dict(common, gwin=gwin))

    res = run_bass_kernel_spmd(nc, in_maps, list(range(8)))

    out = np.zeros((B, T, V), np.float32)
    for c in range(8):
        b, qc = divmod(c, NQ)
        oc = res.results[c]["o"].reshape(P, NQ, K).transpose(1, 0, 2)
        out[b, qc * QF:(qc + 1) * QF, :][:, write_idx] = oc.reshape(QF, K)
    return out
